# revision 2
# baseline (speedup 1.0000x reference)
"""Trainium2 Bass kernel for the CAM factorized-attention module — v3.

v2 was elementwise-bound: exp (ACT) + v-copy (DVE) both sweep [N,512]
through PSUM in phase 1.  v3 removes the v pass entirely using the
identity

    kv_num = E^T v = E^T (x^T Wv) = (E^T x^T) Wv = At^T Wv ,
    At     = [x | 1]^row-major-T  accumulated as  At += xT8o_pair^T E8_pair

where xT8o is the host-packed token-major fp8 copy of x with a ones
column (denominators ride the same accumulation as an extra lhsT row).
Phase 1 then computes only k (not k||v), exp is the single big
elementwise pass, and DVE sits idle — the wall becomes DMA + ACT.

Phase 2 unchanged from v2: pp = M8^T x8 in fp8e5 DoubleRow (M stored
unscaled in e5m2), epilogue = one fused (pp + be) + x16 pass split
across DVE / ACT+DVE / ACT+Pool, fp16 output.

Per-core DMA: x8 3.15 + xT8o 3.16 + x16 6.29 + w 0.5 in, 6.29 out
(~19.4 MB at ~360 GB/s -> ~54 us busy; the wall).
"""

import sys

sys.path.insert(0, "/opt/trn_rl_repo")

import numpy as np
import ml_dtypes

import concourse.bacc as bacc
import concourse.mybir as mybir
from concourse.tile import TileContext
from concourse.bass_utils import run_bass_kernel_spmd

FP32 = mybir.dt.float32
BF16 = mybir.dt.bfloat16
FP16 = mybir.dt.float16
FP8 = mybir.dt.float8e4
FP8E5 = mybir.dt.float8e5
AF = mybir.ActivationFunctionType
DR = mybir.MatmulPerfMode.DoubleRow

C = 256
N = 12288
NCORES = 8
NPAIR = N // 256  # 48 pairs of 128-token chunks
NTILE = N // 768  # 16 k-PSUM tiles of 3 pairs
NG = N // 1024  # 12 phase-2 groups per mt
XTW = 2 * 260  # xT8o flat width per pair (260 = 257 padded to align)

_CACHE = {}


def _build_nc():
    from concourse.alu_op_type import AluOpType

    nc = bacc.Bacc(trn_type="TRN2", target_bir_lowering=False)

    x8_d = nc.declare_dram_parameter("x8", [128, 2, N], FP8, False)
    xt_d = nc.declare_dram_parameter("xt", [128, NPAIR * XTW], FP8, False)
    x16_d = nc.declare_dram_parameter("x16", [128, 2, N], FP16, False)
    wk8_d = nc.declare_dram_parameter("wk8", [128, 2, 256], FP8, False)
    wv_d = nc.declare_dram_parameter("wv", [2, 128, 256], BF16, False)
    wqt_d = nc.declare_dram_parameter("wqt", [2, 128, 256], BF16, False)
    wp_d = nc.declare_dram_parameter("wp", [2, 128, 256], BF16, False)
    bq_d = nc.declare_dram_parameter("bq", [2, 128, 1], BF16, False)
    bp_d = nc.declare_dram_parameter("bp", [2, 128, 1], FP32, False)
    bv_d = nc.declare_dram_parameter("bv", [2, 128, 32], FP32, False)
    id_d = nc.declare_dram_parameter("ident", [128, 128], BF16, False)
    out_d = nc.declare_dram_parameter("out", [2, 128, N], FP16, True)

    with TileContext(nc) as tc:
        with (
            tc.tile_pool(name="const", bufs=1) as const,
            tc.tile_pool(name="resident", bufs=1) as resident,
        ):
            x8 = resident.tile([128, 2, N], FP8, name="x8")
            xt = resident.tile([128, NPAIR * 2, 260], FP8, name="xt")
            x16 = resident.tile([128, 2, N], FP16, name="x16")
            wk8 = const.tile([128, 2, 256], FP8, name="wk8")
            wv = [const.tile([128, 256], BF16, name=f"wv{t}") for t in range(2)]
            wqt = [const.tile([128, 256], BF16, name=f"wqt{t}") for t in range(2)]
            wp = [const.tile([128, 256], BF16, name=f"wp{t}") for t in range(2)]
            bq = [const.tile([128, 1], BF16, name=f"bq{t}") for t in range(2)]
            bp = [const.tile([128, 1], FP32, name=f"bp{t}") for t in range(2)]
            bv = [const.tile([128, 32], FP32, name=f"bv{t}") for t in range(2)]
            Asb = [const.tile([128, 257], BF16, name=f"Asb{t}") for t in range(2)]
            At = [const.tile([128, 256], BF16, name=f"At{ct}") for ct in range(2)]
            ident = const.tile([128, 128], BF16, name="ident")
            kvblk = [const.tile([128, 128], BF16, name=f"kvblk{t}") for t in range(2)]
            Gp = [
                [const.tile([128, 128], BF16, name=f"Gp{t}{kc}") for kc in range(2)]
                for t in range(2)
            ]
            M8 = [const.tile([128, 2, 128], FP8E5, name=f"M8{mt}") for mt in range(2)]
            cq = [const.tile([128, 1], BF16, name=f"cq{t}") for t in range(2)]
            be = [const.tile([128, 1], FP32, name=f"be{mt}") for mt in range(2)]
            recip = [const.tile([128, 1], FP32, name=f"recip{t}") for t in range(2)]

            # phase-1-critical loads: x8/xt chunk-wise (small leading pieces),
            # then x16 in phase-2 consumption order
            nc.sync.dma_start(wk8[:], wk8_d[:, :, :])
            cuts = [0, 256, 768, 1792, 3840, 6656, 9472, N]
            NP16 = 8
            P16 = N // NP16
            xtf = xt[:].rearrange("p a b -> p (a b)")
            x16i = 0
            for i in range(len(cuts) - 1):
                a, b = cuts[i], cuts[i + 1]
                nc.sync.dma_start(x8[:, :, a:b], x8_d[:, :, a:b])
                ta, tb = (a // 256) * XTW, (b // 256) * XTW
                nc.sync.dma_start(xtf[:, ta:tb], xt_d[:, ta:tb])
                # interleave x16 pieces so the stream finishes inside phase 1
                # (only where the x8/xt feed has >2.2us of slack vs exp)
                if False:
                    nc.sync.dma_start(
                        x16[:, :, x16i * P16 : (x16i + 1) * P16],
                        x16_d[:, :, x16i * P16 : (x16i + 1) * P16],
                    )
                    x16i += 1
            for t in range(2):
                nc.sync.dma_start(wv[t][:], wv_d[t])
                nc.sync.dma_start(wqt[t][:], wqt_d[t])
                nc.sync.dma_start(wp[t][:], wp_d[t])
                nc.sync.dma_start(bq[t][:], bq_d[t])
                nc.sync.dma_start(bp[t][:], bp_d[t])
                nc.sync.dma_start(bv[t][:], bv_d[t])
                nc.vector.memset(kvblk[t][:], 0.0)
            nc.sync.dma_start(ident[:], id_d[:, :])
            while x16i < NP16:
                nc.sync.dma_start(
                    x16[:, :, x16i * P16 : (x16i + 1) * P16],
                    x16_d[:, :, x16i * P16 : (x16i + 1) * P16],
                )
                x16i += 1

            # --- phase 1: k, exp, At/S accumulation -------------------------
            with (
                tc.tile_pool(name="accps", bufs=1, space="PSUM") as accps,
                tc.tile_pool(name="kps", bufs=2, space="PSUM") as kps,
                tc.tile_pool(name="ework", bufs=3) as ework,
            ):
                Aps = [
                    accps.tile([128, 257], FP32, name=f"Aps{t}") for t in range(2)
                ]

                for ti in range(NTILE):
                    kp = kps.tile([128, 1536], FP32, name="kp", tag="kp")
                    for half in range(6):
                        n0 = ti * 768 + half * 128
                        nc.tensor.matmul(
                            kp[:, half * 256 : half * 256 + 256],
                            lhsT=x8[:, :, n0 : n0 + 128],
                            rhs=wk8[:],
                            start=True,
                            stop=True,
                            perf_mode=DR,
                        )
                    # E8[p, s, kd] = exp(k), s in 6 half-chunks (3 pairs)
                    E8 = ework.tile([128, 6, 256], FP8, name="E8", tag="E8")
                    nc.scalar.activation(
                        E8[:].rearrange("p s x -> p (s x)"), kp[:], AF.Exp
                    )
                    for j in range(3):
                        pi = ti * 3 + j
                        first, last = pi == 0, pi == NPAIR - 1
                        for t in range(2):
                            nc.tensor.matmul(
                                Aps[t][:],
                                lhsT=E8[:, 2 * j : 2 * j + 2, t * 128 : t * 128 + 128],
                                rhs=xt[:, 2 * pi : 2 * pi + 2, 0:257],
                                start=first,
                                stop=last,
                                perf_mode=DR,
                                skip_group_check=True,
                            )

                # A -> SBUF (bf16); S col is per-partition already: recip
                for t in range(2):
                    nc.scalar.activation(Asb[t][:], Aps[t][:], AF.Identity)
                    nc.vector.reciprocal(recip[t][:], Aps[t][:, 256:257])

            with tc.tile_pool(name="gps", bufs=2, space="PSUM") as gps:
                # transpose A (kd-part, C) -> At (c-part, kd) per 128-block
                for ct in range(2):
                    at_ps = gps.tile([128, 256], BF16, name=f"atps{ct}", tag="tr")
                    for t in range(2):
                        nc.tensor.matmul(
                            at_ps[:, t * 128 : t * 128 + 128],
                            lhsT=Asb[t][:, ct * 128 : ct * 128 + 128],
                            rhs=ident[:],
                            start=True,
                            stop=True,
                            is_transpose=True,
                        )
                    nc.scalar.activation(At[ct][:], at_ps[:], AF.Identity)
                # kv diag-blocks: kvd[t] = sum_ct At[ct][:, t-slice]^T wv[ct][:, t-slice]
                kvd = [gps.tile([128, 128], FP32, name=f"kvd{t}", tag="big") for t in range(2)]
                for t in range(2):
                    for ct in range(2):
                        nc.tensor.matmul(
                            kvd[t][:],
                            lhsT=At[ct][:, t * 128 : t * 128 + 128],
                            rhs=wv[ct][:, t * 128 : t * 128 + 128],
                            start=(ct == 0),
                            stop=(ct == 1),
                        )
                    for g in range(4):
                        r0 = g * 32
                        nc.vector.scalar_tensor_tensor(
                            kvblk[t][r0 : r0 + 32, r0 : r0 + 32],
                            kvd[t][r0 : r0 + 32, r0 : r0 + 32],
                            recip[t][r0 : r0 + 32, :],
                            bv[t][r0 : r0 + 32, :],
                            op0=AluOpType.mult,
                            op1=AluOpType.add,
                        )

                # fold: G = kvblk^T Wq^T, M = G^T Wp' (fp8e5), be.
                # Copies ride ACT (DVE must be free for phase-2 STTs);
                # mt=0 completes first so phase 2 can start on it.
                for t in range(2):
                    cq_ps = gps.tile([128, 1], FP32, name=f"cqps{t}", tag="little")
                    nc.tensor.matmul(
                        cq_ps[:], lhsT=kvblk[t][:], rhs=bq[t][:], start=True, stop=True
                    )
                    nc.scalar.activation(cq[t][:], cq_ps[:], AF.Identity)
                    for kc in range(2):
                        g_ps = gps.tile([128, 128], FP32, name=f"gps{t}{kc}", tag="big")
                        nc.tensor.matmul(
                            g_ps[:],
                            lhsT=kvblk[t][:],
                            rhs=wqt[t][:, kc * 128 : kc * 128 + 128],
                            start=True,
                            stop=True,
                        )
                        nc.scalar.activation(Gp[t][kc][:], g_ps[:], AF.Identity)
                for mt in range(2):
                    for kc in range(2):
                        m_ps = gps.tile([128, 128], FP32, name=f"mps{kc}{mt}", tag="big")
                        for t in range(2):
                            nc.tensor.matmul(
                                m_ps[:],
                                lhsT=Gp[t][kc][:],
                                rhs=wp[t][:, mt * 128 : mt * 128 + 128],
                                start=(t == 0),
                                stop=(t == 1),
                            )
                        nc.scalar.activation(M8[mt][:, kc, :], m_ps[:], AF.Identity)
                    be_ps = gps.tile([128, 1], FP32, name=f"beps{mt}", tag="little")
                    for t in range(2):
                        nc.tensor.matmul(
                            be_ps[:],
                            lhsT=wp[t][:, mt * 128 : mt * 128 + 128],
                            rhs=cq[t][:],
                            start=(t == 0),
                            stop=(t == 1),
                        )
                    nc.scalar.activation(
                        be[mt][:], be_ps[:], AF.Identity, bias=bp[mt][:]
                    )

            # --- phase 2: pp = M8^T x8; out16 = (pp + be) + x16 -------------
            with (
                tc.tile_pool(name="pp_ps", bufs=4, space="PSUM") as pp_ps,
                tc.tile_pool(name="p2out", bufs=12) as p2out,
            ):
                chunks = [(g, mt) for g in range(NG) for mt in range(2)]
                PAT = ["F", "F", "Ad", "Ap", "F", "Ad", "Ap", "F", "Ad",
                       "Ap", "F", "Ad", "Ap", "F", "Ad", "F", "Ap", "Ad",
                       "F", "Ad", "F", "Ap", "F", "Ad"]
                for ci, (g, mt) in enumerate(chunks):
                    n0 = g * 1024
                    pp = pp_ps.tile([128, 1024], FP32, name="pp", tag="pp")
                    for half in range(2):
                        nc.tensor.matmul(
                            pp[:, half * 512 : half * 512 + 512],
                            lhsT=M8[mt][:],
                            rhs=x8[:, :, n0 + half * 512 : n0 + half * 512 + 512],
                            start=True,
                            stop=True,
                            perf_mode=DR,
                        )
                    osb = p2out.tile([128, 1024], FP16, name="osb", tag="osb")
                    xs = x16[:, mt, n0 : n0 + 1024]
                    path = PAT[ci]
                    if path == "F":
                        nc.vector.scalar_tensor_tensor(
                            osb[:], pp[:], be[mt][:], xs,
                            op0=AluOpType.add, op1=AluOpType.add,
                        )
                    else:
                        tmp = p2out.tile([128, 1024], FP16, name="tmp", tag="tmp")
                        nc.scalar.activation(
                            tmp[:], pp[:], AF.Identity, bias=be[mt][:]
                        )
                        if path == "Ap":
                            nc.gpsimd.tensor_add(osb[:], tmp[:], xs)
                        else:
                            nc.vector.tensor_add(osb[:], tmp[:], xs)
                    nc.sync.dma_start(out_d[mt, :, n0 : n0 + 1024], osb[:])
    nc.finalize()
    return nc


def _get_nc():
    if "nc" not in _CACHE:
        _CACHE["nc"] = _build_nc()
    return _CACHE["nc"]


def _prep_in_maps(x, W_qkv, b_qkv, W_proj, b_proj, gamma):
    bf = ml_dtypes.bfloat16
    f8 = ml_dtypes.float8_e4m3
    scale = 32 ** (-0.5)
    g = float(np.asarray(gamma).reshape(-1)[0])

    Wk8 = np.ascontiguousarray(
        W_qkv[:, 256:512].reshape(2, 128, 256).swapaxes(0, 1)
    ).astype(f8)
    Wv = np.ascontiguousarray(W_qkv[:, 512:768].reshape(2, 128, 256)).astype(bf)
    WqT = np.ascontiguousarray(W_qkv[:, 0:256].T.reshape(2, 128, 256)).astype(bf)
    Wp = np.ascontiguousarray((W_proj * (scale * g)).reshape(2, 128, 256)).astype(bf)
    bq = np.ascontiguousarray(b_qkv[0:256].reshape(2, 128, 1)).astype(bf)
    bp = np.ascontiguousarray((g * b_proj).reshape(2, 128, 1)).astype(np.float32)
    bv = np.ascontiguousarray(
        np.broadcast_to(
            b_qkv[512:768].reshape(2, 4, 1, 32), (2, 4, 32, 32)
        ).reshape(2, 128, 32)
    ).astype(np.float32)

    in_maps = []
    for b in range(NCORES):
        xb = np.ascontiguousarray(x[b].reshape(C, N))
        xl = np.ascontiguousarray(xb.reshape(2, 128, N).swapaxes(0, 1))
        # xt[p, 2pi+s, c] = x[c, pi*256 + s*128 + p], col 256 = 1, 257.. pad
        xto = np.zeros((128, NPAIR * 2, 260), np.float32)
        xto[:, :, 256] = 1.0
        xto[:, :, :256] = xb.T.reshape(NPAIR * 2, 128, C).transpose(1, 0, 2)
        in_maps.append(
            {
                "x8": xl.astype(f8),
                "xt": np.ascontiguousarray(
                    xto.reshape(128, NPAIR * XTW)
                ).astype(f8),
                "x16": xl.astype(np.float16),
                "wk8": Wk8,
                "wv": Wv,
                "wqt": WqT,
                "ident": np.eye(128, dtype=ml_dtypes.bfloat16),
                "wp": Wp,
                "bq": bq,
                "bp": bp,
                "bv": bv,
            }
        )
    return in_maps


def kernel(x, W_qkv, b_qkv, W_proj, b_proj, gamma, _trace=False, _trace_kwargs=None):
    x = np.asarray(x, dtype=np.float32)
    nc = _get_nc()
    in_maps = _prep_in_maps(
        x,
        np.asarray(W_qkv, np.float32),
        np.asarray(b_qkv, np.float32),
        np.asarray(W_proj, np.float32),
        np.asarray(b_proj, np.float32),
        np.asarray(gamma, np.float32),
    )
    kw = {}
    if _trace:
        kw = {"trace": True, **(_trace_kwargs or {})}
    res = run_bass_kernel_spmd(nc, in_maps, list(range(NCORES)), **kw)
    out = np.stack(
        [
            res.results[b]["out"].astype(np.float32).reshape(C, 3, 64, 64)
            for b in range(NCORES)
        ]
    )
    if _trace:
        return out, res
    return out


# revision 3
# speedup vs baseline: 1.0460x; 1.0460x over previous
"""Trainium2 Bass kernel for the CAM factorized-attention module — v3.

Data-parallel over B: core b computes batch element b, no collectives.
Same algebraic collapse as the original baseline (the whole attention
branch folds into one [256,256] map M applied to x), rebuilt around the
TimelineSim cost model's economics:

  * All DMA shares ~360 GB/s, so I/O bytes are the wall.  x ships as
    fp8 C-major (matmul operand), fp8 token-major with a ones column
    (kv-statistics operand), and fp16 C-major (residual); the output
    ships as fp16.  End-to-end error ~7e-4 vs the 2e-2 gate.
  * v never exists: kv_num = E^T v = (E^T [x^T|1]) Wv rides the same
    fp8 DoubleRow accumulation that also yields the softmax denominators
    (ones column -> per-partition S).  Phase 1 computes only k and exp.
  * A = E^T[x^T|1] accumulates with E8 as the stationary operand (the
    compiler rejects x^T slices as Ldweights), so a tiny PE transpose
    in the fold flips A into At before kv = At^T Wv.
  * M is stored in fp8e5 WITHOUT a scale (e5m2 spans ~1e-4), which lets
    the whole phase-2 epilogue fuse into one scalar_tensor_tensor per
    chunk: out16 = (pp + be) + x16, split across DVE / ACT+DVE /
    ACT+Pool so no engine exceeds the out-DMA stream.
  * exp writes fp8 E directly; weights load as two blobbed DMAs; x16
    streams behind the phase-1-critical fp8 loads.

Pipeline: phase 1 (16 PSUM tiles of 768 tokens): k = x8^T wk8 (fp8 DR),
E8 = exp(k) (ACT, one op per tile), A[t] += E8_pair^T [xT8|1]_pair.
Fold: recip(S) -> kvblk -> G -> M8 (fp8e5) and be, with copies on ACT.
Phase 2 (24 chunks of 1024 tokens): pp = M8^T x8 (fp8 DR), fused
epilogue, fp16 DMA out.
"""

import sys

sys.path.insert(0, "/opt/trn_rl_repo")

import numpy as np
import ml_dtypes

import concourse.bacc as bacc
import concourse.mybir as mybir
from concourse.tile import TileContext
from concourse.bass_utils import run_bass_kernel_spmd

FP32 = mybir.dt.float32
BF16 = mybir.dt.bfloat16
FP16 = mybir.dt.float16
FP8 = mybir.dt.float8e4
FP8E5 = mybir.dt.float8e5
AF = mybir.ActivationFunctionType
DR = mybir.MatmulPerfMode.DoubleRow

C = 256
N = 12288
NCORES = 8
NPAIR = N // 256  # 48 pairs of 128-token chunks
NTILE = N // 768  # 16 k-PSUM tiles of 3 pairs
NG = N // 1024  # 12 phase-2 groups per mt
XTW = 2 * 260  # xT8o flat width per pair (260 = 257 padded to align)

_CACHE = {}


def _build_nc():
    from concourse.alu_op_type import AluOpType

    nc = bacc.Bacc(trn_type="TRN2", target_bir_lowering=False)

    x8_d = nc.declare_dram_parameter("x8", [128, 2, N], FP8, False)
    xt_d = nc.declare_dram_parameter("xt", [128, NPAIR * XTW], FP8, False)
    x16_d = nc.declare_dram_parameter("x16", [128, 2, N], FP16, False)
    wk8_d = nc.declare_dram_parameter("wk8", [128, 2, 256], FP8, False)
    wb_d = nc.declare_dram_parameter("wb", [2, 128, 769], BF16, False)
    fb_d = nc.declare_dram_parameter("fb", [2, 128, 33], FP32, False)
    id_d = nc.declare_dram_parameter("ident", [128, 128], BF16, False)
    out_d = nc.declare_dram_parameter("out", [2, 128, N], FP16, True)

    with TileContext(nc) as tc:
        with (
            tc.tile_pool(name="const", bufs=1) as const,
            tc.tile_pool(name="resident", bufs=1) as resident,
        ):
            x8 = resident.tile([128, 2, N], FP8, name="x8")
            xt = resident.tile([128, NPAIR * 2, 260], FP8, name="xt")
            x16 = resident.tile([128, 2, N], FP16, name="x16")
            wk8 = const.tile([128, 2, 256], FP8, name="wk8")
            wb = [const.tile([128, 769], BF16, name=f"wb{t}") for t in range(2)]
            fb = [const.tile([128, 33], FP32, name=f"fb{t}") for t in range(2)]
            wv = [wb[t][:, 0:256] for t in range(2)]
            wqt = [wb[t][:, 256:512] for t in range(2)]
            wp = [wb[t][:, 512:768] for t in range(2)]
            bq = [wb[t][:, 768:769] for t in range(2)]
            bp = [fb[t][:, 0:1] for t in range(2)]
            bv = [fb[t][:, 1:33] for t in range(2)]
            Asb = [const.tile([128, 257], BF16, name=f"Asb{t}") for t in range(2)]
            At = [const.tile([128, 256], BF16, name=f"At{ct}") for ct in range(2)]
            ident = const.tile([128, 128], BF16, name="ident")
            kvblk = [const.tile([128, 128], BF16, name=f"kvblk{t}") for t in range(2)]
            Gp = [
                [const.tile([128, 128], BF16, name=f"Gp{t}{kc}") for kc in range(2)]
                for t in range(2)
            ]
            M8 = [const.tile([128, 2, 128], FP8E5, name=f"M8{mt}") for mt in range(2)]
            cq = [const.tile([128, 1], BF16, name=f"cq{t}") for t in range(2)]
            be = [const.tile([128, 1], FP32, name=f"be{mt}") for mt in range(2)]
            recip = [const.tile([128, 1], FP32, name=f"recip{t}") for t in range(2)]

            # phase-1-critical loads: x8/xt chunk-wise (small leading pieces),
            # then x16 in phase-2 consumption order
            nc.sync.dma_start(wk8[:], wk8_d[:, :, :])
            cuts = [0, 256, 768, 1792, 3840, 6656, 9472, N]
            NP16 = 8
            P16 = N // NP16
            xtf = xt[:].rearrange("p a b -> p (a b)")
            x16i = 0
            for i in range(len(cuts) - 1):
                a, b = cuts[i], cuts[i + 1]
                nc.sync.dma_start(x8[:, :, a:b], x8_d[:, :, a:b])
                ta, tb = (a // 256) * XTW, (b // 256) * XTW
                nc.sync.dma_start(xtf[:, ta:tb], xt_d[:, ta:tb])
                # interleave x16 pieces so the stream finishes inside phase 1
                # (only where the x8/xt feed has >2.2us of slack vs exp)
                if False:
                    nc.sync.dma_start(
                        x16[:, :, x16i * P16 : (x16i + 1) * P16],
                        x16_d[:, :, x16i * P16 : (x16i + 1) * P16],
                    )
                    x16i += 1
            for t in range(2):
                nc.sync.dma_start(wb[t][:], wb_d[t])
                nc.sync.dma_start(fb[t][:], fb_d[t])
                nc.vector.memset(kvblk[t][:], 0.0)
            nc.sync.dma_start(ident[:], id_d[:, :])
            while x16i < NP16:
                nc.sync.dma_start(
                    x16[:, :, x16i * P16 : (x16i + 1) * P16],
                    x16_d[:, :, x16i * P16 : (x16i + 1) * P16],
                )
                x16i += 1

            # --- phase 1: k, exp, At/S accumulation -------------------------
            with (
                tc.tile_pool(name="accps", bufs=1, space="PSUM") as accps,
                tc.tile_pool(name="kps", bufs=2, space="PSUM") as kps,
                tc.tile_pool(name="ework", bufs=3) as ework,
            ):
                Aps = [
                    accps.tile([128, 257], FP32, name=f"Aps{t}") for t in range(2)
                ]

                for ti in range(NTILE):
                    kp = kps.tile([128, 1536], FP32, name="kp", tag="kp")
                    for half in range(6):
                        n0 = ti * 768 + half * 128
                        nc.tensor.matmul(
                            kp[:, half * 256 : half * 256 + 256],
                            lhsT=x8[:, :, n0 : n0 + 128],
                            rhs=wk8[:],
                            start=True,
                            stop=True,
                            perf_mode=DR,
                        )
                    # E8[p, s, kd] = exp(k), s in 6 half-chunks (3 pairs)
                    E8 = ework.tile([128, 6, 256], FP8, name="E8", tag="E8")
                    nc.scalar.activation(
                        E8[:].rearrange("p s x -> p (s x)"), kp[:], AF.Exp
                    )
                    for j in range(3):
                        pi = ti * 3 + j
                        first, last = pi == 0, pi == NPAIR - 1
                        for t in range(2):
                            nc.tensor.matmul(
                                Aps[t][:],
                                lhsT=E8[:, 2 * j : 2 * j + 2, t * 128 : t * 128 + 128],
                                rhs=xt[:, 2 * pi : 2 * pi + 2, 0:257],
                                start=first,
                                stop=last,
                                perf_mode=DR,
                                skip_group_check=True,
                            )

                # A -> SBUF (bf16); S col is per-partition already: recip
                for t in range(2):
                    nc.scalar.activation(Asb[t][:], Aps[t][:], AF.Identity)
                    nc.vector.reciprocal(recip[t][:], Aps[t][:, 256:257])

            with tc.tile_pool(name="gps", bufs=2, space="PSUM") as gps:
                # transpose A (kd-part, C) -> At (c-part, kd) per 128-block
                for ct in range(2):
                    at_ps = gps.tile([128, 256], BF16, name=f"atps{ct}", tag="tr")
                    for t in range(2):
                        nc.tensor.matmul(
                            at_ps[:, t * 128 : t * 128 + 128],
                            lhsT=Asb[t][:, ct * 128 : ct * 128 + 128],
                            rhs=ident[:],
                            start=True,
                            stop=True,
                            is_transpose=True,
                        )
                    nc.scalar.activation(At[ct][:], at_ps[:], AF.Identity)
                # kv diag-blocks: kvd[t] = sum_ct At[ct][:, t-slice]^T wv[ct][:, t-slice]
                kvd = [gps.tile([128, 128], FP32, name=f"kvd{t}", tag="big") for t in range(2)]
                for t in range(2):
                    for ct in range(2):
                        nc.tensor.matmul(
                            kvd[t][:],
                            lhsT=At[ct][:, t * 128 : t * 128 + 128],
                            rhs=wv[ct][:, t * 128 : t * 128 + 128],
                            start=(ct == 0),
                            stop=(ct == 1),
                        )
                    for g in range(4):
                        r0 = g * 32
                        nc.vector.scalar_tensor_tensor(
                            kvblk[t][r0 : r0 + 32, r0 : r0 + 32],
                            kvd[t][r0 : r0 + 32, r0 : r0 + 32],
                            recip[t][r0 : r0 + 32, :],
                            bv[t][r0 : r0 + 32, :],
                            op0=AluOpType.mult,
                            op1=AluOpType.add,
                        )

                # fold: G = kvblk^T Wq^T, M = G^T Wp' (fp8e5), be.
                # Copies ride ACT (DVE must be free for phase-2 STTs);
                # mt=0 completes first so phase 2 can start on it.
                for t in range(2):
                    cq_ps = gps.tile([128, 1], FP32, name=f"cqps{t}", tag="little")
                    nc.tensor.matmul(
                        cq_ps[:], lhsT=kvblk[t][:], rhs=bq[t], start=True, stop=True
                    )
                    nc.scalar.activation(cq[t][:], cq_ps[:], AF.Identity)
                    for kc in range(2):
                        g_ps = gps.tile([128, 128], FP32, name=f"gps{t}{kc}", tag="big")
                        nc.tensor.matmul(
                            g_ps[:],
                            lhsT=kvblk[t][:],
                            rhs=wqt[t][:, kc * 128 : kc * 128 + 128],
                            start=True,
                            stop=True,
                        )
                        nc.scalar.activation(Gp[t][kc][:], g_ps[:], AF.Identity)
                for mt in range(2):
                    for kc in range(2):
                        m_ps = gps.tile([128, 128], FP32, name=f"mps{kc}{mt}", tag="big")
                        for t in range(2):
                            nc.tensor.matmul(
                                m_ps[:],
                                lhsT=Gp[t][kc][:],
                                rhs=wp[t][:, mt * 128 : mt * 128 + 128],
                                start=(t == 0),
                                stop=(t == 1),
                            )
                        nc.scalar.activation(M8[mt][:, kc, :], m_ps[:], AF.Identity)
                    be_ps = gps.tile([128, 1], FP32, name=f"beps{mt}", tag="little")
                    for t in range(2):
                        nc.tensor.matmul(
                            be_ps[:],
                            lhsT=wp[t][:, mt * 128 : mt * 128 + 128],
                            rhs=cq[t][:],
                            start=(t == 0),
                            stop=(t == 1),
                        )
                    nc.scalar.activation(
                        be[mt][:], be_ps[:], AF.Identity, bias=bp[mt][:]
                    )

            # --- phase 2: pp = M8^T x8; out16 = (pp + be) + x16 -------------
            with (
                tc.tile_pool(name="pp_ps", bufs=4, space="PSUM") as pp_ps,
                tc.tile_pool(name="p2out", bufs=12) as p2out,
            ):
                chunks = [(g, mt) for g in range(NG) for mt in range(2)]
                PAT = ["F", "F", "Ad", "Ap", "F", "Ad", "Ap", "F", "Ad",
                       "Ap", "F", "Ad", "Ap", "F", "Ad", "F", "Ap", "Ad",
                       "F", "Ad", "F", "Ap", "F", "Ad"]
                for ci, (g, mt) in enumerate(chunks):
                    n0 = g * 1024
                    pp = pp_ps.tile([128, 1024], FP32, name="pp", tag="pp")
                    for half in range(2):
                        nc.tensor.matmul(
                            pp[:, half * 512 : half * 512 + 512],
                            lhsT=M8[mt][:],
                            rhs=x8[:, :, n0 + half * 512 : n0 + half * 512 + 512],
                            start=True,
                            stop=True,
                            perf_mode=DR,
                        )
                    osb = p2out.tile([128, 1024], FP16, name="osb", tag="osb")
                    xs = x16[:, mt, n0 : n0 + 1024]
                    path = PAT[ci]
                    if path == "F":
                        nc.vector.scalar_tensor_tensor(
                            osb[:], pp[:], be[mt][:], xs,
                            op0=AluOpType.add, op1=AluOpType.add,
                        )
                    else:
                        tmp = p2out.tile([128, 1024], FP16, name="tmp", tag="tmp")
                        nc.scalar.activation(
                            tmp[:], pp[:], AF.Identity, bias=be[mt][:]
                        )
                        if path == "Ap":
                            nc.gpsimd.tensor_add(osb[:], tmp[:], xs)
                        else:
                            nc.vector.tensor_add(osb[:], tmp[:], xs)
                    nc.sync.dma_start(out_d[mt, :, n0 : n0 + 1024], osb[:])
    nc.finalize()
    return nc


def _get_nc():
    if "nc" not in _CACHE:
        _CACHE["nc"] = _build_nc()
    return _CACHE["nc"]


def _prep_in_maps(x, W_qkv, b_qkv, W_proj, b_proj, gamma):
    bf = ml_dtypes.bfloat16
    f8 = ml_dtypes.float8_e4m3
    scale = 32 ** (-0.5)
    g = float(np.asarray(gamma).reshape(-1)[0])

    Wk8 = np.ascontiguousarray(
        W_qkv[:, 256:512].reshape(2, 128, 256).swapaxes(0, 1)
    ).astype(f8)
    wb = np.zeros((2, 128, 769), bf)
    wb[:, :, 0:256] = W_qkv[:, 512:768].reshape(2, 128, 256).astype(bf)
    wb[:, :, 256:512] = W_qkv[:, 0:256].T.reshape(2, 128, 256).astype(bf)
    wb[:, :, 512:768] = (W_proj * (scale * g)).reshape(2, 128, 256).astype(bf)
    wb[:, :, 768:769] = b_qkv[0:256].reshape(2, 128, 1).astype(bf)
    fbb = np.zeros((2, 128, 33), np.float32)
    fbb[:, :, 0:1] = (g * b_proj).reshape(2, 128, 1)
    fbb[:, :, 1:33] = np.broadcast_to(
        b_qkv[512:768].reshape(2, 4, 1, 32), (2, 4, 32, 32)
    ).reshape(2, 128, 32)

    in_maps = []
    for b in range(NCORES):
        xb = np.ascontiguousarray(x[b].reshape(C, N))
        xl = np.ascontiguousarray(xb.reshape(2, 128, N).swapaxes(0, 1))
        # xt[p, 2pi+s, c] = x[c, pi*256 + s*128 + p], col 256 = 1, 257.. pad
        xto = np.zeros((128, NPAIR * 2, 260), np.float32)
        xto[:, :, 256] = 1.0
        xto[:, :, :256] = xb.T.reshape(NPAIR * 2, 128, C).transpose(1, 0, 2)
        in_maps.append(
            {
                "x8": xl.astype(f8),
                "xt": np.ascontiguousarray(
                    xto.reshape(128, NPAIR * XTW)
                ).astype(f8),
                "x16": xl.astype(np.float16),
                "wk8": Wk8,
                "wb": wb,
                "fb": fbb,
                "ident": np.eye(128, dtype=ml_dtypes.bfloat16),
            }
        )
    return in_maps


def kernel(x, W_qkv, b_qkv, W_proj, b_proj, gamma, _trace=False, _trace_kwargs=None):
    x = np.asarray(x, dtype=np.float32)
    nc = _get_nc()
    in_maps = _prep_in_maps(
        x,
        np.asarray(W_qkv, np.float32),
        np.asarray(b_qkv, np.float32),
        np.asarray(W_proj, np.float32),
        np.asarray(b_proj, np.float32),
        np.asarray(gamma, np.float32),
    )
    kw = {}
    if _trace:
        kw = {"trace": True, **(_trace_kwargs or {})}
    res = run_bass_kernel_spmd(nc, in_maps, list(range(NCORES)), **kw)
    out = np.stack(
        [
            res.results[b]["out"].astype(np.float32).reshape(C, 3, 64, 64)
            for b in range(NCORES)
        ]
    )
    if _trace:
        return out, res
    return out
